# revision 32
# baseline (speedup 1.0000x reference)
"""Trainium2 Bass kernel for batch-8 multi-head self-attention with
contiguous-span masking (B=8, N=2048, DIN=DM=256, NH=4, DK=64).

Sharding: data-parallel over batch — core b computes sample b end-to-end.

Per-core dataflow (everything kept transposed, feature-on-partition, so all
softmax reductions are along the free axis and no PE transposes are needed):

  xT [256, 2048]  --Wq/Wk-->  QT_aug/KT_aug  (4 head tiles of [65, 2048]):
      rows 0..63 = head projection, row 64 = valid_i (QT) / vbias_j (KT).
  S^T[j, i] = sum_{d<64} KT[d,j]*QT[d,i] + vbias_j*valid_i + (-1e10)*inval_i
      (K=66 matmul — the additive span mask is folded into the contraction
      for free; rows 64/65 of the augmented operands hold the mask vectors).
  P = exp(0.125 * S^T)  — no max subtraction needed: unmasked scores are
      O(1) and masked scores are ~-1.25e9 so exp underflows to exactly 0.
      The fp32 reference's -1e10 bias ABSORBS the scores (ulp(1e10)=1024),
      making every padding-row score exactly -1e10 and hence its softmax
      exactly uniform; the same absorption happens in our PSUM accumulation,
      and the uniform result is restored by the vbar rank-1 fix below.
  U^T[d', i] = sum_j V_aug[j, d'] * P[j, i]  with V_aug[:, 64] = 1, so row 64
      accumulates the softmax denominator alongside the 64 value rows.
      One extra rank-1 matmul adds vbar_aug[d'] * inval_i, where
      vbar_aug = [mean_j V, 1.0]: padding columns get U = mean(V), sum = 1.
  attT = U^T[0:64] * (1 / U^T[64])  (DVE reciprocal + broadcast multiply)
  outT[e, i] = sum_d Wo[d, e] * attT[d, i] + bo[e]

Matmuls run as float32r (1 cycle/row vs 4 for fp32); flip USE_F32R off for
full-fp32 accuracy at ~3x the runtime.
"""

import numpy as np

import concourse.bass as bass
import concourse.mybir as mybir
from concourse import bacc, bass_utils
from concourse.tile import TileContext


B, N, DIN, DM, NH, DK = 8, 2048, 256, 256, 4, 64
SCALE = 1.0 / 8.0  # 1/sqrt(DK)
NEG = -1e10

F32 = mybir.dt.float32
BF16 = mybir.dt.bfloat16
IC = 512  # i-chunk width (matmul moving-operand cap for fp32)
NI = N // IC  # 4 i-chunks
NJ = N // 128  # 16 j-chunks
EG = 3  # j-chunks per exp group (3 PSUM banks per S^T group buffer)
DKP = DK + 2  # V_aug columns: 64 values + denominator ones + even-pad
# (fp32r matmuls require even innermost free counts; col 65 is a dummy)

USE_F32R = True


QK_BF16 = True  # Q/K operands in bf16: 1 cyc/row vs 2 for fp32r (adds ~3e-4 err)
PV_BF16 = True  # P/V operands in bf16: halves PV matmul time (adds ~2e-3 err)


def _emit(nc, tc, d):
    MM = mybir.dt.float32r if USE_F32R else F32
    QK = BF16 if QK_BF16 else MM
    PV = BF16 if PV_BF16 else MM
    Exp = mybir.ActivationFunctionType.Exp

    with (
        tc.tile_pool(name="consts", bufs=1) as consts,
        tc.tile_pool(name="persist", bufs=1) as persist,
    ):
        # ---- persistent attention operands --------------------------------
        xT = [persist.tile([128, N], MM, tag=f"xT{c}", name=f"xT{c}") for c in range(2)]
        qT = [persist.tile([66, N], QK, tag=f"qT{h}", name=f"qT{h}") for h in range(NH)]
        kT = [persist.tile([66, N], QK, tag=f"kT{h}", name=f"kT{h}") for h in range(NH)]
        vA = [persist.tile([128, NH, DKP], PV, tag=f"vA{j}", name=f"vA{j}") for j in range(NJ)]
        vbar = [consts.tile([1, DKP], PV, tag=f"vbar{h}", name=f"vbar{h}") for h in range(NH)]

        wq, wk, wv, wo = [], [], [], []
        bqk, bo_sb = [], []
        for c in range(2):
            for lst, name in ((wq, "Wq"), (wk, "Wk"), (wv, "Wv"), (wo, "Wo")):
                lst.append(consts.tile([128, DM], MM, tag=f"{name}_r{c}", name=f"{name}_r{c}"))
            bqk.append(consts.tile([128, 2], F32, tag=f"bqk{c}", name=f"bqk{c}"))
            bo_sb.append(consts.tile([128, 1], F32, tag=f"bo{c}", name=f"bo{c}"))
        bv_r = consts.tile([1, DM], F32, tag="bv_r", name="bv_r")
        bv_bc = consts.tile([128, NH, DK], F32, tag="bv_bc", name="bv_bc")
        inval_r = consts.tile([1, N], PV, tag="inval_r", name="inval_r")
        ones_col = consts.tile([1, 128], MM, tag="ones_col", name="ones_col")
        vones = consts.tile([128, NH, 2], F32, tag="vones", name="vones")
        nc.vector.memset(vones, 1.0)
        ones128 = consts.tile([128, 2], PV, tag="ones128", name="ones128")

        # ---- load + round everything (staging pool closes afterwards) -----
        with tc.tile_pool(name="stage", bufs=2) as stage:
            def load_row(name, dst):
                w = dst.shape[-1]
                s = stage.tile([1, N], F32, tag="rowstage", name="rowstage")
                nc.sync.dma_start(out=s[0:1, 0:w], in_=d[name][0:1, 0:w])
                nc.vector.tensor_copy(dst, s[0:1, 0:w])

            def load_w(lst, name, c, act=False):
                s = stage.tile([128, DM], F32, tag="wstage", name="wstage")
                nc.sync.dma_start(out=s, in_=d[name][c * 128 : (c + 1) * 128, :])
                if act:
                    nc.scalar.copy(lst[c], s)
                else:
                    nc.vector.tensor_copy(lst[c], s)

            def load_x(c, i):
                isl = bass.ts(i, IC)
                s = stage.tile([128, IC], F32, tag="xstage", name="xstage")
                nc.sync.dma_start(out=s, in_=d["xT"][c * 128 : (c + 1) * 128, isl])
                if c == 1:
                    nc.scalar.copy(xT[c][:, isl], s)
                else:
                    nc.vector.tensor_copy(xT[c][:, isl], s)

            # critical-path order: Wk + x slice 0 unblock the first K-proj
            for c in range(2):
                load_w(wk, "Wk", c)
            for c in range(2):
                load_x(c, 0)
            for c in range(2):
                load_w(wq, "Wq", c)
                nc.sync.dma_start(out=bqk[c], in_=d["bqk"][c * 128 : (c + 1) * 128, :])
            for i in range(1, NI):
                for c in range(2):
                    load_x(c, i)
            qrs = stage.tile([2, N], F32, tag="qrs", name="qrs")
            nc.sync.dma_start(out=qrs, in_=d["qrows"][:, :])
            krs = stage.tile([2, N], F32, tag="krs", name="krs")
            nc.sync.dma_start(out=krs, in_=d["krows"][:, :])
            for h in range(NH):
                nc.vector.tensor_copy(qT[h][64:66, :], qrs)
                nc.vector.tensor_copy(kT[h][64:66, :], krs)
            for c in range(2):
                load_w(wv, "Wv", c, act=True)
                load_w(wo, "Wo", c, act=True)
                nc.sync.dma_start(out=bo_sb[c], in_=d["bo"][c * 128 : (c + 1) * 128, :])

            s = stage.tile([1, N], F32, tag="rowstage", name="rowstage")
            nc.sync.dma_start(out=s[0:1, 0:DM], in_=d["bv"][0:1, :])
            nc.vector.tensor_copy(bv_r, s[0:1, 0:DM])
            nc.gpsimd.partition_broadcast(
                bv_bc[:, :, :].rearrange("p h k -> p (h k)"), bv_r
            )
            load_row("inval", inval_r)

            ones_stage = stage.tile([1, 128], F32, tag="ones", name="ones")
            nc.vector.memset(ones_stage, 1.0)
            nc.vector.tensor_copy(ones_col, ones_stage)
            o128s = stage.tile([128, 2], F32, tag="o128s", name="o128s")
            nc.vector.memset(o128s, 1.0)
            nc.vector.tensor_copy(ones128, o128s)


        groups = [list(range(g, min(g + EG, NJ))) for g in range(0, NJ, EG)]
        with (
            tc.tile_pool(name="psA", bufs=2, space="PSUM") as psA,
            tc.tile_pool(name="psS", bufs=2, space="PSUM") as psS,
            tc.tile_pool(name="expS", bufs=2) as expP,
            tc.tile_pool(name="nrm", bufs=2) as nrm,
            tc.tile_pool(name="attP", bufs=2) as attP,
            tc.tile_pool(name="outP", bufs=2) as outP,
        ):
            # ---- K then Q projections, i-outer so slice 0 unblocks fast ---
            def proj_kq(ws, i, col):
                isl = bass.ts(i, IC)
                for m in range(2):
                    p = psA.tile([128, IC], F32, tag="proj", name="proj")
                    for c in range(2):
                        nc.tensor.matmul(
                            p,
                            lhsT=ws[c][:, m * 128 : (m + 1) * 128],
                            rhs=xT[c][:, isl],
                            start=(c == 0),
                            stop=(c == 1),
                        )
                    dst = kT if col else qT
                    for hh in range(2):
                        h = 2 * m + hh
                        nc.vector.tensor_scalar_add(
                            dst[h][0:64, isl],
                            p[hh * 64 : (hh + 1) * 64, :],
                            bqk[m][hh * 64 : (hh + 1) * 64, col : col + 1],
                        )

            for i in range(NI):
                proj_kq(wk, i, 1)
                proj_kq(wq, i, 0)
            for j in range(NJ):
                p = psA.tile([128, DM], F32, tag="proj", name="proj")
                jsl = bass.ts(j, 128)
                for c in range(2):
                    nc.tensor.matmul(
                        p,
                        lhsT=xT[c][:, jsl],
                        rhs=wv[c],
                        start=(c == 0),
                        stop=(c == 1),
                    )
                nc.vector.tensor_tensor(
                    vA[j][:, :, 0:DK],
                    p[:, :].rearrange("p (h k) -> p h k", h=NH),
                    bv_bc,
                    op=mybir.AluOpType.add,
                )
                nc.vector.tensor_copy(vA[j][:, :, DK:DKP], vones)

            # ---- vbar_aug = [mean_j V_h, 1.0] per head (uniform-row fix) --
            vp = psA.tile([2, NH, DKP], F32, tag="proj", name="vbarp")
            for j in range(NJ):
                nc.tensor.matmul(
                    vp[:, :, :],
                    lhsT=ones128,
                    rhs=vA[j][:, :, :],
                    start=(j == 0),
                    stop=(j == NJ - 1),
                )
            for h in range(NH):
                nc.vector.tensor_scalar_mul(vbar[h], vp[0:1, h, :], 1.0 / N)


            # ---- attention + output projection ----------------------------
            def out_proj(i, attT):
                isl = bass.ts(i, IC)
                for e in range(2):
                    p = psA.tile([128, IC], F32, tag="proj", name="outp")
                    for c in range(2):
                        nc.tensor.matmul(
                            p,
                            lhsT=wo[c][:, e * 128 : (e + 1) * 128],
                            rhs=attT[c],
                            start=(c == 0),
                            stop=(c == 1),
                        )
                    o = outP.tile([128, IC], F32, tag="out", name="out")
                    nc.vector.tensor_scalar_add(o, p, bo_sb[e])
                    nc.sync.dma_start(
                        out=d["outT"][e * 128 : (e + 1) * 128, isl], in_=o
                    )

            pending = None
            for i in range(NI):
                isl = bass.ts(i, IC)
                attT = [attP.tile([128, IC], MM, tag=f"attT{c}", name=f"attT{c}") for c in range(2)]
                for h in range(NH):
                    up = psA.tile([66, IC], F32, tag="proj", name="U")
                    for grp in groups:
                        g = len(grp)
                        sp = psS.tile([128, EG, IC], F32, tag="S", name="S")
                        for gg, j in enumerate(grp):
                            nc.tensor.matmul(
                                sp[:, gg, :],
                                lhsT=kT[h][:, bass.ts(j, 128)],
                                rhs=qT[h][:, isl],
                                start=True,
                                stop=True,
                            )
                        e = expP.tile([128, EG, IC], PV, tag="expS", name="expS")
                        nc.scalar.activation(
                            e[:, 0:g, :], sp[:, 0:g, :], Exp, scale=SCALE
                        )
                        for gg, j in enumerate(grp):
                            nc.tensor.matmul(
                                up,
                                lhsT=vA[j][:, h, :],
                                rhs=e[:, gg, :],
                                start=(j == 0),
                                stop=False,
                            )
                    nc.tensor.matmul(
                        up,
                        lhsT=vbar[h],
                        rhs=inval_r[0:1, isl],
                        start=False,
                        stop=True,
                    )
                    rsum = nrm.tile([1, IC], F32, tag="rsum", name="rsum")
                    nc.vector.tensor_copy(rsum, up[64:65, :])
                    rec = nrm.tile([1, IC], F32, tag="rec", name="rec")
                    nc.vector.reciprocal_approx_fast(rec, rsum)
                    bc = nrm.tile([64, IC], F32, tag="bc", name="bc")
                    nc.gpsimd.partition_broadcast(bc, rec[0:1, :])
                    nc.vector.tensor_mul(
                        attT[h // 2][(h % 2) * 64 : (h % 2 + 1) * 64, :],
                        up[0:64, :],
                        bc,
                    )
                if pending is not None:
                    out_proj(*pending)
                pending = (i, attT)
            out_proj(*pending)


_NC_CACHE = {}


def _build():
    key = USE_F32R
    if key in _NC_CACHE:
        return _NC_CACHE[key]
    nc = bacc.Bacc("TRN2", debug=False, num_devices=B)
    d = {
        "xT": nc.dram_tensor("xT", [DIN, N], F32, kind="ExternalInput").ap(),
        "Wq": nc.dram_tensor("Wq", [DIN, DM], F32, kind="ExternalInput").ap(),
        "Wk": nc.dram_tensor("Wk", [DIN, DM], F32, kind="ExternalInput").ap(),
        "Wv": nc.dram_tensor("Wv", [DIN, DM], F32, kind="ExternalInput").ap(),
        "Wo": nc.dram_tensor("Wo", [DM, DM], F32, kind="ExternalInput").ap(),
        "bqk": nc.dram_tensor("bqk", [DM, 2], F32, kind="ExternalInput").ap(),
        "bv": nc.dram_tensor("bv", [1, DM], F32, kind="ExternalInput").ap(),
        "bo": nc.dram_tensor("bo", [DM, 1], F32, kind="ExternalInput").ap(),
        "qrows": nc.dram_tensor("qrows", [2, N], F32, kind="ExternalInput").ap(),
        "krows": nc.dram_tensor("krows", [2, N], F32, kind="ExternalInput").ap(),
        "inval": nc.dram_tensor("inval", [1, N], F32, kind="ExternalInput").ap(),
        "outT": nc.dram_tensor("outT", [DM, N], F32, kind="ExternalOutput").ap(),
    }
    with TileContext(nc) as tc:
        _emit(nc, tc, d)
    nc.compile()
    _NC_CACHE[key] = nc
    return nc


def _host_marshal(x, attention_mask, Wq, bq, Wk, bk, Wv, bv, Wo, bo):
    x = np.asarray(x, dtype=np.float32)
    m = np.asarray(attention_mask).astype(bool)
    pos = np.arange(N)
    start = m.argmax(axis=1)  # first True index
    end = N - 1 - m[:, ::-1].argmax(axis=1)  # last True index (exclusive bound)
    valid = (pos[None, :] >= start[:, None]) & (pos[None, :] < end[:, None])
    valid_f = valid.astype(np.float32)
    vbias_f = np.where(valid, np.float32(0.0), np.float32(NEG)).astype(np.float32)

    common = {
        "Wq": np.ascontiguousarray(Wq, dtype=np.float32),
        "Wk": np.ascontiguousarray(Wk, dtype=np.float32),
        "Wv": np.ascontiguousarray(Wv, dtype=np.float32),
        "Wo": np.ascontiguousarray(Wo, dtype=np.float32),
        "bqk": np.ascontiguousarray(
            np.stack([np.asarray(bq), np.asarray(bk)], axis=1), dtype=np.float32
        ),
        "bv": np.asarray(bv, dtype=np.float32).reshape(1, DM),
        "bo": np.asarray(bo, dtype=np.float32).reshape(DM, 1),
    }
    in_maps = []
    for b in range(B):
        im = dict(common)
        im["xT"] = np.ascontiguousarray(x[b].T)
        inval = np.float32(1.0) - valid_f[b : b + 1]
        im["qrows"] = np.concatenate([valid_f[b : b + 1], inval], axis=0)
        im["krows"] = np.concatenate(
            [vbias_f[b : b + 1], np.full((1, N), NEG, dtype=np.float32)], axis=0
        )
        im["inval"] = inval
        in_maps.append(im)
    return in_maps


def kernel(x, attention_mask, Wq, bq, Wk, bk, Wv, bv, Wo, bo, _trace=False):
    nc = _build()
    in_maps = _host_marshal(x, attention_mask, Wq, bq, Wk, bk, Wv, bv, Wo, bo)
    res = bass_utils.run_bass_kernel_spmd(
        nc, in_maps, core_ids=list(range(B)), trace=_trace
    )
    out = np.stack([np.ascontiguousarray(r["outT"].T) for r in res.results], axis=0)
    if _trace:
        kernel.last_exec_time_ns = res.exec_time_ns
        kernel.last_results = res
    return out


# revision 33
# speedup vs baseline: 1.3462x; 1.3462x over previous
"""Trainium2 Bass kernel for batch-8 multi-head self-attention with
contiguous-span masking (B=8, N=2048, DIN=DM=256, NH=4, DK=64).

Sharding: data-parallel over batch — core b computes sample b end-to-end.

Per-core dataflow (everything kept transposed, feature-on-partition, so all
softmax reductions are along the free axis and no PE transposes are needed):

  xT [256, 2048]  --Wq/Wk-->  QT_aug/KT_aug  (4 head tiles of [65, 2048]):
      rows 0..63 = head projection, row 64 = valid_i (QT) / vbias_j (KT).
  S^T[j, i] = sum_{d<64} KT[d,j]*QT[d,i] + vbias_j*valid_i + (-1e10)*inval_i
      (K=66 matmul — the additive span mask is folded into the contraction
      for free; rows 64/65 of the augmented operands hold the mask vectors).
  P = exp(0.125 * S^T)  — no max subtraction needed: unmasked scores are
      O(1) and masked scores are ~-1.25e9 so exp underflows to exactly 0.
      The fp32 reference's -1e10 bias ABSORBS the scores (ulp(1e10)=1024),
      making every padding-row score exactly -1e10 and hence its softmax
      exactly uniform; the same absorption happens in our PSUM accumulation,
      and the uniform result is restored by the vbar rank-1 fix below.
  U^T[d', i] = sum_j V_aug[j, d'] * P[j, i]  with V_aug[:, 64] = 1, so row 64
      accumulates the softmax denominator alongside the 64 value rows.
      One extra rank-1 matmul adds vbar_aug[d'] * inval_i, where
      vbar_aug = [mean_j V, 1.0]: padding columns get U = mean(V), sum = 1.
  attT = U^T[0:64] * (1 / U^T[64])  (DVE reciprocal + broadcast multiply)
  outT[e, i] = sum_d Wo[d, e] * attT[d, i] + bo[e]

Matmuls run as float32r (1 cycle/row vs 4 for fp32); flip USE_F32R off for
full-fp32 accuracy at ~3x the runtime.
"""

import numpy as np

import concourse.bass as bass
import concourse.mybir as mybir
from concourse import bacc, bass_utils
from concourse.tile import TileContext


B, N, DIN, DM, NH, DK = 8, 2048, 256, 256, 4, 64
SCALE = 1.0 / 8.0  # 1/sqrt(DK)
NEG = -1e10

F32 = mybir.dt.float32
BF16 = mybir.dt.bfloat16
IC = 512  # i-chunk width (matmul moving-operand cap for fp32)
NI = N // IC  # 4 i-chunks
NJ = N // 128  # 16 j-chunks
EG = 2  # j-chunks per exp group (2 PSUM banks per S^T group buffer)
DKP = DK + 2  # V_aug columns: 64 values + denominator ones + even-pad
# (fp32r matmuls require even innermost free counts; col 65 is a dummy)

USE_F32R = True


QK_BF16 = True  # Q/K operands in bf16: 1 cyc/row vs 2 for fp32r (adds ~3e-4 err)
PV_BF16 = True  # P/V operands in bf16: halves PV matmul time (adds ~2e-3 err)


def _emit(nc, tc, d):
    MM = mybir.dt.float32r if USE_F32R else F32
    QK = BF16 if QK_BF16 else MM
    PV = BF16 if PV_BF16 else MM
    Exp = mybir.ActivationFunctionType.Exp

    with (
        tc.tile_pool(name="consts", bufs=1) as consts,
        tc.tile_pool(name="persist", bufs=1) as persist,
    ):
        # ---- persistent attention operands --------------------------------
        xT = [persist.tile([128, N], MM, tag=f"xT{c}", name=f"xT{c}") for c in range(2)]
        qT = [persist.tile([66, N], QK, tag=f"qT{h}", name=f"qT{h}") for h in range(NH)]
        kT = [persist.tile([66, N], QK, tag=f"kT{h}", name=f"kT{h}") for h in range(NH)]
        vA = [persist.tile([128, NH, DKP], PV, tag=f"vA{j}", name=f"vA{j}") for j in range(NJ)]
        vbar = [consts.tile([1, DKP], PV, tag=f"vbar{h}", name=f"vbar{h}") for h in range(NH)]

        wq, wk, wv, wo = [], [], [], []
        bqk, bo_sb = [], []
        for c in range(2):
            for lst, name in ((wq, "Wq"), (wk, "Wk"), (wv, "Wv"), (wo, "Wo")):
                lst.append(consts.tile([128, DM], MM, tag=f"{name}_r{c}", name=f"{name}_r{c}"))
            bqk.append(consts.tile([128, 2], F32, tag=f"bqk{c}", name=f"bqk{c}"))
            bo_sb.append(consts.tile([128, 1], F32, tag=f"bo{c}", name=f"bo{c}"))
        bv_r = consts.tile([1, DM], F32, tag="bv_r", name="bv_r")
        bv_bc = consts.tile([128, NH, DK], F32, tag="bv_bc", name="bv_bc")
        inval_r = consts.tile([1, N], PV, tag="inval_r", name="inval_r")
        ones_col = consts.tile([1, 128], MM, tag="ones_col", name="ones_col")
        vones = consts.tile([128, NH, 2], F32, tag="vones", name="vones")
        nc.vector.memset(vones, 1.0)
        ones128 = consts.tile([128, 2], PV, tag="ones128", name="ones128")

        # ---- load + round everything (staging pool closes afterwards) -----
        with tc.tile_pool(name="stage", bufs=2) as stage:
            def load_row(name, dst):
                w = dst.shape[-1]
                s = stage.tile([1, N], F32, tag="rowstage", name="rowstage")
                nc.sync.dma_start(out=s[0:1, 0:w], in_=d[name][0:1, 0:w])
                nc.vector.tensor_copy(dst, s[0:1, 0:w])

            def load_w(lst, name, c, act=False):
                s = stage.tile([128, DM], F32, tag="wstage", name="wstage")
                nc.sync.dma_start(out=s, in_=d[name][c * 128 : (c + 1) * 128, :])
                if act:
                    nc.scalar.copy(lst[c], s)
                else:
                    nc.vector.tensor_copy(lst[c], s)

            def load_x(c, i):
                isl = bass.ts(i, IC)
                s = stage.tile([128, IC], F32, tag="xstage", name="xstage")
                nc.sync.dma_start(out=s, in_=d["xT"][c * 128 : (c + 1) * 128, isl])
                if c == 1:
                    nc.scalar.copy(xT[c][:, isl], s)
                else:
                    nc.vector.tensor_copy(xT[c][:, isl], s)

            # critical-path order: Wk + x slice 0 unblock the first K-proj
            for c in range(2):
                load_w(wk, "Wk", c)
            for c in range(2):
                load_x(c, 0)
            for c in range(2):
                load_w(wq, "Wq", c)
                nc.sync.dma_start(out=bqk[c], in_=d["bqk"][c * 128 : (c + 1) * 128, :])
            for i in range(1, NI):
                for c in range(2):
                    load_x(c, i)
            qrs = stage.tile([2, N], F32, tag="qrs", name="qrs")
            nc.sync.dma_start(out=qrs, in_=d["qrows"][:, :])
            krs = stage.tile([2, N], F32, tag="krs", name="krs")
            nc.sync.dma_start(out=krs, in_=d["krows"][:, :])
            for h in range(NH):
                nc.vector.tensor_copy(qT[h][64:66, :], qrs)
                nc.vector.tensor_copy(kT[h][64:66, :], krs)
            for c in range(2):
                load_w(wv, "Wv", c, act=True)
                load_w(wo, "Wo", c, act=True)
                nc.sync.dma_start(out=bo_sb[c], in_=d["bo"][c * 128 : (c + 1) * 128, :])

            s = stage.tile([1, N], F32, tag="rowstage", name="rowstage")
            nc.sync.dma_start(out=s[0:1, 0:DM], in_=d["bv"][0:1, :])
            nc.vector.tensor_copy(bv_r, s[0:1, 0:DM])
            nc.gpsimd.partition_broadcast(
                bv_bc[:, :, :].rearrange("p h k -> p (h k)"), bv_r
            )
            load_row("inval", inval_r)

            ones_stage = stage.tile([1, 128], F32, tag="ones", name="ones")
            nc.vector.memset(ones_stage, 1.0)
            nc.vector.tensor_copy(ones_col, ones_stage)
            o128s = stage.tile([128, 2], F32, tag="o128s", name="o128s")
            nc.vector.memset(o128s, 1.0)
            nc.vector.tensor_copy(ones128, o128s)


        groups = [list(range(g, min(g + EG, NJ))) for g in range(0, NJ, EG)]
        with (
            tc.tile_pool(name="psA", bufs=2, space="PSUM") as psA,
            tc.tile_pool(name="psS", bufs=3, space="PSUM") as psS,
            tc.tile_pool(name="expS", bufs=2) as expP,
            tc.tile_pool(name="nrm", bufs=2) as nrm,
            tc.tile_pool(name="attP", bufs=2) as attP,
            tc.tile_pool(name="outP", bufs=2) as outP,
        ):
            # ---- K then Q projections, i-outer so slice 0 unblocks fast ---
            def proj_kq(ws, i, col):
                isl = bass.ts(i, IC)
                for m in range(2):
                    p = psA.tile([128, IC], F32, tag="proj", name="proj")
                    for c in range(2):
                        nc.tensor.matmul(
                            p,
                            lhsT=ws[c][:, m * 128 : (m + 1) * 128],
                            rhs=xT[c][:, isl],
                            start=(c == 0),
                            stop=(c == 1),
                        )
                    dst = kT if col else qT
                    for hh in range(2):
                        h = 2 * m + hh
                        nc.vector.tensor_scalar_add(
                            dst[h][0:64, isl],
                            p[hh * 64 : (hh + 1) * 64, :],
                            bqk[m][hh * 64 : (hh + 1) * 64, col : col + 1],
                        )

            for i in range(NI):
                proj_kq(wk, i, 1)
                proj_kq(wq, i, 0)
            for j in range(NJ):
                p = psA.tile([128, DM], F32, tag="proj", name="proj")
                jsl = bass.ts(j, 128)
                for c in range(2):
                    nc.tensor.matmul(
                        p,
                        lhsT=xT[c][:, jsl],
                        rhs=wv[c],
                        start=(c == 0),
                        stop=(c == 1),
                    )
                nc.vector.tensor_tensor(
                    vA[j][:, :, 0:DK],
                    p[:, :].rearrange("p (h k) -> p h k", h=NH),
                    bv_bc,
                    op=mybir.AluOpType.add,
                )
                nc.vector.tensor_copy(vA[j][:, :, DK:DKP], vones)

            # ---- vbar_aug = [mean_j V_h, 1.0] per head (uniform-row fix) --
            vp = psA.tile([2, NH, DKP], F32, tag="proj", name="vbarp")
            for j in range(NJ):
                nc.tensor.matmul(
                    vp[:, :, :],
                    lhsT=ones128,
                    rhs=vA[j][:, :, :],
                    start=(j == 0),
                    stop=(j == NJ - 1),
                )
            for h in range(NH):
                nc.vector.tensor_scalar_mul(vbar[h], vp[0:1, h, :], 1.0 / N)


            # ---- attention + output projection ----------------------------
            def out_proj(i, attT):
                isl = bass.ts(i, IC)
                for e in range(2):
                    p = psA.tile([128, IC], F32, tag="proj", name="outp")
                    for c in range(2):
                        nc.tensor.matmul(
                            p,
                            lhsT=wo[c][:, e * 128 : (e + 1) * 128],
                            rhs=attT[c],
                            start=(c == 0),
                            stop=(c == 1),
                        )
                    o = outP.tile([128, IC], F32, tag="out", name="out")
                    nc.vector.tensor_scalar_add(o, p, bo_sb[e])
                    nc.sync.dma_start(
                        out=d["outT"][e * 128 : (e + 1) * 128, isl], in_=o
                    )

            pending = None
            for i in range(NI):
                isl = bass.ts(i, IC)
                attT = [attP.tile([128, IC], MM, tag=f"attT{c}", name=f"attT{c}") for c in range(2)]
                for h in range(NH):
                    up = psA.tile([66, IC], F32, tag="proj", name="U")
                    for grp in groups:
                        g = len(grp)
                        sp = psS.tile([128, EG, IC], F32, tag="S", name="S")
                        for gg, j in enumerate(grp):
                            nc.tensor.matmul(
                                sp[:, gg, :],
                                lhsT=kT[h][:, bass.ts(j, 128)],
                                rhs=qT[h][:, isl],
                                start=True,
                                stop=True,
                            )
                        e = expP.tile([128, EG, IC], PV, tag="expS", name="expS")
                        nc.scalar.activation(
                            e[:, 0:g, :], sp[:, 0:g, :], Exp, scale=SCALE
                        )
                        for gg, j in enumerate(grp):
                            nc.tensor.matmul(
                                up,
                                lhsT=vA[j][:, h, :],
                                rhs=e[:, gg, :],
                                start=(j == 0),
                                stop=False,
                            )
                    nc.tensor.matmul(
                        up,
                        lhsT=vbar[h],
                        rhs=inval_r[0:1, isl],
                        start=False,
                        stop=True,
                    )
                    rsum = nrm.tile([1, IC], F32, tag="rsum", name="rsum")
                    nc.vector.tensor_copy(rsum, up[64:65, :])
                    rec = nrm.tile([1, IC], F32, tag="rec", name="rec")
                    nc.vector.reciprocal_approx_fast(rec, rsum)
                    bc = nrm.tile([64, IC], F32, tag="bc", name="bc")
                    nc.gpsimd.partition_broadcast(bc, rec[0:1, :])
                    nc.vector.tensor_mul(
                        attT[h // 2][(h % 2) * 64 : (h % 2 + 1) * 64, :],
                        up[0:64, :],
                        bc,
                    )
                if pending is not None:
                    out_proj(*pending)
                pending = (i, attT)
            out_proj(*pending)


_NC_CACHE = {}


def _build():
    key = USE_F32R
    if key in _NC_CACHE:
        return _NC_CACHE[key]
    nc = bacc.Bacc("TRN2", debug=False, num_devices=B)
    d = {
        "xT": nc.dram_tensor("xT", [DIN, N], F32, kind="ExternalInput").ap(),
        "Wq": nc.dram_tensor("Wq", [DIN, DM], F32, kind="ExternalInput").ap(),
        "Wk": nc.dram_tensor("Wk", [DIN, DM], F32, kind="ExternalInput").ap(),
        "Wv": nc.dram_tensor("Wv", [DIN, DM], F32, kind="ExternalInput").ap(),
        "Wo": nc.dram_tensor("Wo", [DM, DM], F32, kind="ExternalInput").ap(),
        "bqk": nc.dram_tensor("bqk", [DM, 2], F32, kind="ExternalInput").ap(),
        "bv": nc.dram_tensor("bv", [1, DM], F32, kind="ExternalInput").ap(),
        "bo": nc.dram_tensor("bo", [DM, 1], F32, kind="ExternalInput").ap(),
        "qrows": nc.dram_tensor("qrows", [2, N], F32, kind="ExternalInput").ap(),
        "krows": nc.dram_tensor("krows", [2, N], F32, kind="ExternalInput").ap(),
        "inval": nc.dram_tensor("inval", [1, N], F32, kind="ExternalInput").ap(),
        "outT": nc.dram_tensor("outT", [DM, N], F32, kind="ExternalOutput").ap(),
    }
    with TileContext(nc) as tc:
        _emit(nc, tc, d)
    nc.compile()
    _NC_CACHE[key] = nc
    return nc


def _host_marshal(x, attention_mask, Wq, bq, Wk, bk, Wv, bv, Wo, bo):
    x = np.asarray(x, dtype=np.float32)
    m = np.asarray(attention_mask).astype(bool)
    pos = np.arange(N)
    start = m.argmax(axis=1)  # first True index
    end = N - 1 - m[:, ::-1].argmax(axis=1)  # last True index (exclusive bound)
    valid = (pos[None, :] >= start[:, None]) & (pos[None, :] < end[:, None])
    valid_f = valid.astype(np.float32)
    vbias_f = np.where(valid, np.float32(0.0), np.float32(NEG)).astype(np.float32)

    common = {
        "Wq": np.ascontiguousarray(Wq, dtype=np.float32),
        "Wk": np.ascontiguousarray(Wk, dtype=np.float32),
        "Wv": np.ascontiguousarray(Wv, dtype=np.float32),
        "Wo": np.ascontiguousarray(Wo, dtype=np.float32),
        "bqk": np.ascontiguousarray(
            np.stack([np.asarray(bq), np.asarray(bk)], axis=1), dtype=np.float32
        ),
        "bv": np.asarray(bv, dtype=np.float32).reshape(1, DM),
        "bo": np.asarray(bo, dtype=np.float32).reshape(DM, 1),
    }
    in_maps = []
    for b in range(B):
        im = dict(common)
        im["xT"] = np.ascontiguousarray(x[b].T)
        inval = np.float32(1.0) - valid_f[b : b + 1]
        im["qrows"] = np.concatenate([valid_f[b : b + 1], inval], axis=0)
        im["krows"] = np.concatenate(
            [vbias_f[b : b + 1], np.full((1, N), NEG, dtype=np.float32)], axis=0
        )
        im["inval"] = inval
        in_maps.append(im)
    return in_maps


def kernel(x, attention_mask, Wq, bq, Wk, bk, Wv, bv, Wo, bo, _trace=False):
    nc = _build()
    in_maps = _host_marshal(x, attention_mask, Wq, bq, Wk, bk, Wv, bv, Wo, bo)
    res = bass_utils.run_bass_kernel_spmd(
        nc, in_maps, core_ids=list(range(B)), trace=_trace
    )
    out = np.stack([np.ascontiguousarray(r["outT"].T) for r in res.results], axis=0)
    if _trace:
        kernel.last_exec_time_ns = res.exec_time_ns
        kernel.last_results = res
    return out


# revision 34
# speedup vs baseline: 1.4021x; 1.0415x over previous
"""Trainium2 Bass kernel for batch-8 multi-head self-attention with
contiguous-span masking (B=8, N=2048, DIN=DM=256, NH=4, DK=64).

Sharding: data-parallel over batch — core b computes sample b end-to-end.

Per-core dataflow (everything kept transposed, feature-on-partition, so all
softmax reductions are along the free axis and no PE transposes are needed):

  xT [256, 2048]  --Wq/Wk-->  QT_aug/KT_aug  (4 head tiles of [65, 2048]):
      rows 0..63 = head projection, row 64 = valid_i (QT) / vbias_j (KT).
  S^T[j, i] = sum_{d<64} KT[d,j]*QT[d,i] + vbias_j*valid_i + (-1e10)*inval_i
      (K=66 matmul — the additive span mask is folded into the contraction
      for free; rows 64/65 of the augmented operands hold the mask vectors).
  P = exp(0.125 * S^T)  — no max subtraction needed: unmasked scores are
      O(1) and masked scores are ~-1.25e9 so exp underflows to exactly 0.
      The fp32 reference's -1e10 bias ABSORBS the scores (ulp(1e10)=1024),
      making every padding-row score exactly -1e10 and hence its softmax
      exactly uniform; the same absorption happens in our PSUM accumulation,
      and the uniform result is restored by the vbar rank-1 fix below.
  U^T[d', i] = sum_j V_aug[j, d'] * P[j, i]  with V_aug[:, 64] = 1, so row 64
      accumulates the softmax denominator alongside the 64 value rows.
      One extra rank-1 matmul adds vbar_aug[d'] * inval_i, where
      vbar_aug = [mean_j V, 1.0]: padding columns get U = mean(V), sum = 1.
  attT = U^T[0:64] * (1 / U^T[64])  (DVE reciprocal + broadcast multiply)
  outT[e, i] = sum_d Wo[d, e] * attT[d, i] + bo[e]

Matmuls run as float32r (1 cycle/row vs 4 for fp32); flip USE_F32R off for
full-fp32 accuracy at ~3x the runtime.
"""

import numpy as np

import concourse.bass as bass
import concourse.mybir as mybir
from concourse import bacc, bass_utils
from concourse.tile import TileContext


B, N, DIN, DM, NH, DK = 8, 2048, 256, 256, 4, 64
SCALE = 1.0 / 8.0  # 1/sqrt(DK)
NEG = -1e10

F32 = mybir.dt.float32
BF16 = mybir.dt.bfloat16
IC = 512  # i-chunk width (matmul moving-operand cap for fp32)
NI = N // IC  # 4 i-chunks
NJ = N // 128  # 16 j-chunks
EG = 2  # j-chunks per exp group (2 PSUM banks per S^T group buffer)
DKP = DK + 2  # V_aug columns: 64 values + denominator ones + even-pad
# (fp32r matmuls require even innermost free counts; col 65 is a dummy)

USE_F32R = True


QK_BF16 = True  # Q/K operands in bf16: 1 cyc/row vs 2 for fp32r (adds ~3e-4 err)
PV_BF16 = True  # P/V operands in bf16: halves PV matmul time (adds ~2e-3 err)


def _emit(nc, tc, d):
    MM = mybir.dt.float32r if USE_F32R else F32
    QK = BF16 if QK_BF16 else MM
    PV = BF16 if PV_BF16 else MM
    Exp = mybir.ActivationFunctionType.Exp

    with (
        tc.tile_pool(name="consts", bufs=1) as consts,
        tc.tile_pool(name="persist", bufs=1) as persist,
    ):
        # ---- persistent attention operands --------------------------------
        xT = [persist.tile([128, N], MM, tag=f"xT{c}", name=f"xT{c}") for c in range(2)]
        qT = [persist.tile([66, N], QK, tag=f"qT{h}", name=f"qT{h}") for h in range(NH)]
        kT = [persist.tile([66, N], QK, tag=f"kT{h}", name=f"kT{h}") for h in range(NH)]
        vA = [persist.tile([128, NH, DKP], PV, tag=f"vA{j}", name=f"vA{j}") for j in range(NJ)]
        vbar = [consts.tile([1, DKP], PV, tag=f"vbar{h}", name=f"vbar{h}") for h in range(NH)]

        wq, wk, wv, wo = [], [], [], []
        bqk, bo_sb = [], []
        for c in range(2):
            for lst, name in ((wq, "Wq"), (wk, "Wk"), (wv, "Wv"), (wo, "Wo")):
                lst.append(consts.tile([128, DM], MM, tag=f"{name}_r{c}", name=f"{name}_r{c}"))
            bqk.append(consts.tile([128, 2], F32, tag=f"bqk{c}", name=f"bqk{c}"))
            bo_sb.append(consts.tile([128, 1], F32, tag=f"bo{c}", name=f"bo{c}"))
        bv_r = consts.tile([1, DM], F32, tag="bv_r", name="bv_r")
        bv_bc = consts.tile([128, NH, DK], F32, tag="bv_bc", name="bv_bc")
        inval_r = consts.tile([1, N], PV, tag="inval_r", name="inval_r")
        ones_col = consts.tile([1, 128], MM, tag="ones_col", name="ones_col")
        vones = consts.tile([128, NH, 2], F32, tag="vones", name="vones")
        nc.vector.memset(vones, 1.0)
        ones128 = consts.tile([128, 2], PV, tag="ones128", name="ones128")

        # ---- load + round everything (staging pool closes afterwards) -----
        with tc.tile_pool(name="stage", bufs=2) as stage:
            def load_row(name, dst):
                w = dst.shape[-1]
                s = stage.tile([1, N], F32, tag="rowstage", name="rowstage")
                nc.sync.dma_start(out=s[0:1, 0:w], in_=d[name][0:1, 0:w])
                nc.vector.tensor_copy(dst, s[0:1, 0:w])

            def load_w(lst, name, c, act=False):
                s = stage.tile([128, DM], F32, tag="wstage", name="wstage")
                nc.sync.dma_start(out=s, in_=d[name][c * 128 : (c + 1) * 128, :])
                if act:
                    nc.scalar.copy(lst[c], s)
                else:
                    nc.vector.tensor_copy(lst[c], s)

            def load_x(c, i):
                isl = bass.ts(i, IC)
                s = stage.tile([128, IC], F32, tag="xstage", name="xstage")
                nc.sync.dma_start(out=s, in_=d["xT"][c * 128 : (c + 1) * 128, isl])
                if c == 1:
                    nc.scalar.copy(xT[c][:, isl], s)
                else:
                    nc.vector.tensor_copy(xT[c][:, isl], s)

            # critical-path order: Wk + x slice 0 unblock the first K-proj
            for c in range(2):
                load_w(wk, "Wk", c)
            for c in range(2):
                load_x(c, 0)
            for c in range(2):
                load_w(wq, "Wq", c)
                nc.sync.dma_start(out=bqk[c], in_=d["bqk"][c * 128 : (c + 1) * 128, :])
            for i in range(1, NI):
                for c in range(2):
                    load_x(c, i)
            qrs = stage.tile([2, N], F32, tag="qrs", name="qrs")
            nc.sync.dma_start(out=qrs, in_=d["qrows"][:, :])
            krs = stage.tile([2, N], F32, tag="krs", name="krs")
            nc.sync.dma_start(out=krs, in_=d["krows"][:, :])
            for h in range(NH):
                nc.vector.tensor_copy(qT[h][64:66, :], qrs)
                nc.vector.tensor_copy(kT[h][64:66, :], krs)
            for c in range(2):
                load_w(wv, "Wv", c, act=True)
                load_w(wo, "Wo", c, act=True)
                nc.sync.dma_start(out=bo_sb[c], in_=d["bo"][c * 128 : (c + 1) * 128, :])

            s = stage.tile([1, N], F32, tag="rowstage", name="rowstage")
            nc.sync.dma_start(out=s[0:1, 0:DM], in_=d["bv"][0:1, :])
            nc.vector.tensor_copy(bv_r, s[0:1, 0:DM])
            nc.gpsimd.partition_broadcast(
                bv_bc[:, :, :].rearrange("p h k -> p (h k)"), bv_r
            )
            load_row("inval", inval_r)

            ones_stage = stage.tile([1, 128], F32, tag="ones", name="ones")
            nc.vector.memset(ones_stage, 1.0)
            nc.vector.tensor_copy(ones_col, ones_stage)
            o128s = stage.tile([128, 2], F32, tag="o128s", name="o128s")
            nc.vector.memset(o128s, 1.0)
            nc.vector.tensor_copy(ones128, o128s)


        groups = [list(range(g, min(g + EG, NJ))) for g in range(0, NJ, EG)]
        with (
            tc.tile_pool(name="psA", bufs=2, space="PSUM") as psA,
            tc.tile_pool(name="psS", bufs=3, space="PSUM") as psS,
            tc.tile_pool(name="expS", bufs=2) as expP,
            tc.tile_pool(name="nrm", bufs=2) as nrm,
            tc.tile_pool(name="attP", bufs=2) as attP,
            tc.tile_pool(name="outP", bufs=2) as outP,
        ):
            # ---- K then Q projections, i-outer so slice 0 unblocks fast ---
            def proj_kq(ws, i, col):
                isl = bass.ts(i, IC)
                for m in range(2):
                    p = psA.tile([128, IC], F32, tag="proj", name="proj")
                    for c in range(2):
                        nc.tensor.matmul(
                            p,
                            lhsT=ws[c][:, m * 128 : (m + 1) * 128],
                            rhs=xT[c][:, isl],
                            start=(c == 0),
                            stop=(c == 1),
                        )
                    dst = kT if col else qT
                    for hh in range(2):
                        h = 2 * m + hh
                        if hh:
                            nc.scalar.activation(
                                dst[h][0:64, isl],
                                p[hh * 64 : (hh + 1) * 64, :],
                                mybir.ActivationFunctionType.Identity,
                                bias=bqk[m][hh * 64 : (hh + 1) * 64, col : col + 1],
                            )
                        else:
                            nc.vector.tensor_scalar_add(
                                dst[h][0:64, isl],
                                p[hh * 64 : (hh + 1) * 64, :],
                                bqk[m][hh * 64 : (hh + 1) * 64, col : col + 1],
                            )

            for i in range(NI):
                proj_kq(wk, i, 1)
                proj_kq(wq, i, 0)
            for j in range(NJ):
                p = psA.tile([128, DM], F32, tag="proj", name="proj")
                jsl = bass.ts(j, 128)
                for c in range(2):
                    nc.tensor.matmul(
                        p,
                        lhsT=xT[c][:, jsl],
                        rhs=wv[c],
                        start=(c == 0),
                        stop=(c == 1),
                    )
                nc.vector.tensor_tensor(
                    vA[j][:, :, 0:DK],
                    p[:, :].rearrange("p (h k) -> p h k", h=NH),
                    bv_bc,
                    op=mybir.AluOpType.add,
                )
                nc.vector.tensor_copy(vA[j][:, :, DK:DKP], vones)

            # ---- vbar_aug = [mean_j V_h, 1.0] per head (uniform-row fix) --
            vp = psA.tile([2, NH, DKP], F32, tag="proj", name="vbarp")
            for j in range(NJ):
                nc.tensor.matmul(
                    vp[:, :, :],
                    lhsT=ones128,
                    rhs=vA[j][:, :, :],
                    start=(j == 0),
                    stop=(j == NJ - 1),
                )
            for h in range(NH):
                nc.vector.tensor_scalar_mul(vbar[h], vp[0:1, h, :], 1.0 / N)


            # ---- attention + output projection ----------------------------
            def out_proj(i, attT):
                isl = bass.ts(i, IC)
                for e in range(2):
                    p = psA.tile([128, IC], F32, tag="proj", name="outp")
                    for c in range(2):
                        nc.tensor.matmul(
                            p,
                            lhsT=wo[c][:, e * 128 : (e + 1) * 128],
                            rhs=attT[c],
                            start=(c == 0),
                            stop=(c == 1),
                        )
                    o = outP.tile([128, IC], F32, tag="out", name="out")
                    nc.vector.tensor_scalar_add(o, p, bo_sb[e])
                    nc.sync.dma_start(
                        out=d["outT"][e * 128 : (e + 1) * 128, isl], in_=o
                    )

            pending = None
            for i in range(NI):
                isl = bass.ts(i, IC)
                attT = [attP.tile([128, IC], MM, tag=f"attT{c}", name=f"attT{c}") for c in range(2)]
                for h in range(NH):
                    up = psA.tile([66, IC], F32, tag="proj", name="U")
                    for grp in groups:
                        g = len(grp)
                        sp = psS.tile([128, EG, IC], F32, tag="S", name="S")
                        for gg, j in enumerate(grp):
                            nc.tensor.matmul(
                                sp[:, gg, :],
                                lhsT=kT[h][:, bass.ts(j, 128)],
                                rhs=qT[h][:, isl],
                                start=True,
                                stop=True,
                            )
                        e = expP.tile([128, EG, IC], PV, tag="expS", name="expS")
                        nc.scalar.activation(
                            e[:, 0:g, :], sp[:, 0:g, :], Exp, scale=SCALE
                        )
                        for gg, j in enumerate(grp):
                            nc.tensor.matmul(
                                up,
                                lhsT=vA[j][:, h, :],
                                rhs=e[:, gg, :],
                                start=(j == 0),
                                stop=False,
                            )
                    nc.tensor.matmul(
                        up,
                        lhsT=vbar[h],
                        rhs=inval_r[0:1, isl],
                        start=False,
                        stop=True,
                    )
                    rsum = nrm.tile([1, IC], F32, tag="rsum", name="rsum")
                    nc.vector.tensor_copy(rsum, up[64:65, :])
                    rec = nrm.tile([1, IC], F32, tag="rec", name="rec")
                    nc.vector.reciprocal_approx_fast(rec, rsum)
                    bc = nrm.tile([64, IC], F32, tag="bc", name="bc")
                    nc.gpsimd.partition_broadcast(bc, rec[0:1, :])
                    nc.vector.tensor_mul(
                        attT[h // 2][(h % 2) * 64 : (h % 2 + 1) * 64, :],
                        up[0:64, :],
                        bc,
                    )
                if pending is not None:
                    out_proj(*pending)
                pending = (i, attT)
            out_proj(*pending)


_NC_CACHE = {}


def _build():
    key = USE_F32R
    if key in _NC_CACHE:
        return _NC_CACHE[key]
    nc = bacc.Bacc("TRN2", debug=False, num_devices=B)
    d = {
        "xT": nc.dram_tensor("xT", [DIN, N], F32, kind="ExternalInput").ap(),
        "Wq": nc.dram_tensor("Wq", [DIN, DM], F32, kind="ExternalInput").ap(),
        "Wk": nc.dram_tensor("Wk", [DIN, DM], F32, kind="ExternalInput").ap(),
        "Wv": nc.dram_tensor("Wv", [DIN, DM], F32, kind="ExternalInput").ap(),
        "Wo": nc.dram_tensor("Wo", [DM, DM], F32, kind="ExternalInput").ap(),
        "bqk": nc.dram_tensor("bqk", [DM, 2], F32, kind="ExternalInput").ap(),
        "bv": nc.dram_tensor("bv", [1, DM], F32, kind="ExternalInput").ap(),
        "bo": nc.dram_tensor("bo", [DM, 1], F32, kind="ExternalInput").ap(),
        "qrows": nc.dram_tensor("qrows", [2, N], F32, kind="ExternalInput").ap(),
        "krows": nc.dram_tensor("krows", [2, N], F32, kind="ExternalInput").ap(),
        "inval": nc.dram_tensor("inval", [1, N], F32, kind="ExternalInput").ap(),
        "outT": nc.dram_tensor("outT", [DM, N], F32, kind="ExternalOutput").ap(),
    }
    with TileContext(nc) as tc:
        _emit(nc, tc, d)
    nc.compile()
    _NC_CACHE[key] = nc
    return nc


def _host_marshal(x, attention_mask, Wq, bq, Wk, bk, Wv, bv, Wo, bo):
    x = np.asarray(x, dtype=np.float32)
    m = np.asarray(attention_mask).astype(bool)
    pos = np.arange(N)
    start = m.argmax(axis=1)  # first True index
    end = N - 1 - m[:, ::-1].argmax(axis=1)  # last True index (exclusive bound)
    valid = (pos[None, :] >= start[:, None]) & (pos[None, :] < end[:, None])
    valid_f = valid.astype(np.float32)
    vbias_f = np.where(valid, np.float32(0.0), np.float32(NEG)).astype(np.float32)

    common = {
        "Wq": np.ascontiguousarray(Wq, dtype=np.float32),
        "Wk": np.ascontiguousarray(Wk, dtype=np.float32),
        "Wv": np.ascontiguousarray(Wv, dtype=np.float32),
        "Wo": np.ascontiguousarray(Wo, dtype=np.float32),
        "bqk": np.ascontiguousarray(
            np.stack([np.asarray(bq), np.asarray(bk)], axis=1), dtype=np.float32
        ),
        "bv": np.asarray(bv, dtype=np.float32).reshape(1, DM),
        "bo": np.asarray(bo, dtype=np.float32).reshape(DM, 1),
    }
    in_maps = []
    for b in range(B):
        im = dict(common)
        im["xT"] = np.ascontiguousarray(x[b].T)
        inval = np.float32(1.0) - valid_f[b : b + 1]
        im["qrows"] = np.concatenate([valid_f[b : b + 1], inval], axis=0)
        im["krows"] = np.concatenate(
            [vbias_f[b : b + 1], np.full((1, N), NEG, dtype=np.float32)], axis=0
        )
        im["inval"] = inval
        in_maps.append(im)
    return in_maps


def kernel(x, attention_mask, Wq, bq, Wk, bk, Wv, bv, Wo, bo, _trace=False):
    nc = _build()
    in_maps = _host_marshal(x, attention_mask, Wq, bq, Wk, bk, Wv, bv, Wo, bo)
    res = bass_utils.run_bass_kernel_spmd(
        nc, in_maps, core_ids=list(range(B)), trace=_trace
    )
    out = np.stack([np.ascontiguousarray(r["outT"].T) for r in res.results], axis=0)
    if _trace:
        kernel.last_exec_time_ns = res.exec_time_ns
        kernel.last_results = res
    return out


# revision 35
# speedup vs baseline: 1.4389x; 1.0263x over previous
"""Trainium2 Bass kernel for batch-8 multi-head self-attention with
contiguous-span masking (B=8, N=2048, DIN=DM=256, NH=4, DK=64).

Sharding: data-parallel over batch — core b computes sample b end-to-end.

Per-core dataflow (everything kept transposed, feature-on-partition, so all
softmax reductions are along the free axis and no PE transposes are needed):

  xT [256, 2048]  --Wq/Wk-->  QT_aug/KT_aug  (4 head tiles of [65, 2048]):
      rows 0..63 = head projection, row 64 = valid_i (QT) / vbias_j (KT).
  S^T[j, i] = sum_{d<64} KT[d,j]*QT[d,i] + vbias_j*valid_i + (-1e10)*inval_i
      (K=66 matmul — the additive span mask is folded into the contraction
      for free; rows 64/65 of the augmented operands hold the mask vectors).
  P = exp(0.125 * S^T)  — no max subtraction needed: unmasked scores are
      O(1) and masked scores are ~-1.25e9 so exp underflows to exactly 0.
      The fp32 reference's -1e10 bias ABSORBS the scores (ulp(1e10)=1024),
      making every padding-row score exactly -1e10 and hence its softmax
      exactly uniform; the same absorption happens in our PSUM accumulation,
      and the uniform result is restored by the vbar rank-1 fix below.
  U^T[d', i] = sum_j V_aug[j, d'] * P[j, i]  with V_aug[:, 64] = 1, so row 64
      accumulates the softmax denominator alongside the 64 value rows.
      One extra rank-1 matmul adds vbar_aug[d'] * inval_i, where
      vbar_aug = [mean_j V, 1.0]: padding columns get U = mean(V), sum = 1.
  attT = U^T[0:64] * (1 / U^T[64])  (DVE reciprocal + broadcast multiply)
  outT[e, i] = sum_d Wo[d, e] * attT[d, i] + bo[e]

Matmuls run as float32r (1 cycle/row vs 4 for fp32); flip USE_F32R off for
full-fp32 accuracy at ~3x the runtime.
"""

import numpy as np

import concourse.bass as bass
import concourse.mybir as mybir
from concourse import bacc, bass_utils
from concourse.tile import TileContext


B, N, DIN, DM, NH, DK = 8, 2048, 256, 256, 4, 64
SCALE = 1.0 / 8.0  # 1/sqrt(DK)
NEG = -1e10

F32 = mybir.dt.float32
BF16 = mybir.dt.bfloat16
IC = 512  # i-chunk width (matmul moving-operand cap for fp32)
NI = N // IC  # 4 i-chunks
NJ = N // 128  # 16 j-chunks
EG = 2  # j-chunks per exp group (2 PSUM banks per S^T group buffer)
DKP = DK + 2  # V_aug columns: 64 values + denominator ones + even-pad
# (fp32r matmuls require even innermost free counts; col 65 is a dummy)

USE_F32R = True


QK_BF16 = True  # Q/K operands in bf16: 1 cyc/row vs 2 for fp32r (adds ~3e-4 err)
PV_BF16 = True  # P/V operands in bf16: halves PV matmul time (adds ~2e-3 err)


def _emit(nc, tc, d):
    MM = mybir.dt.float32r if USE_F32R else F32
    QK = BF16 if QK_BF16 else MM
    PV = BF16 if PV_BF16 else MM
    Exp = mybir.ActivationFunctionType.Exp

    with (
        tc.tile_pool(name="consts", bufs=1) as consts,
        tc.tile_pool(name="persist", bufs=1) as persist,
    ):
        # ---- persistent attention operands --------------------------------
        xT = [persist.tile([128, N], QK, tag=f"xT{c}", name=f"xT{c}") for c in range(2)]
        qT = [persist.tile([66, N], QK, tag=f"qT{h}", name=f"qT{h}") for h in range(NH)]
        kT = [persist.tile([66, N], QK, tag=f"kT{h}", name=f"kT{h}") for h in range(NH)]
        vA = [persist.tile([128, NH, DKP], PV, tag=f"vA{j}", name=f"vA{j}") for j in range(NJ)]
        vbar = [consts.tile([1, DKP], PV, tag=f"vbar{h}", name=f"vbar{h}") for h in range(NH)]

        wq, wk, wv, wo = [], [], [], []
        bqk, bo_sb = [], []
        for c in range(2):
            for lst, name, dt_ in (
                (wq, "Wq", QK),
                (wk, "Wk", QK),
                (wv, "Wv", PV),
                (wo, "Wo", MM),
            ):
                lst.append(consts.tile([128, DM], dt_, tag=f"{name}_r{c}", name=f"{name}_r{c}"))
            bqk.append(consts.tile([128, 2], F32, tag=f"bqk{c}", name=f"bqk{c}"))
            bo_sb.append(consts.tile([128, 1], F32, tag=f"bo{c}", name=f"bo{c}"))
        bv_r = consts.tile([1, DM], F32, tag="bv_r", name="bv_r")
        bv_bc = consts.tile([128, NH, DK], F32, tag="bv_bc", name="bv_bc")
        inval_r = consts.tile([1, N], PV, tag="inval_r", name="inval_r")
        ones_col = consts.tile([1, 128], MM, tag="ones_col", name="ones_col")
        vones = consts.tile([128, NH, 2], F32, tag="vones", name="vones")
        nc.vector.memset(vones, 1.0)
        ones128 = consts.tile([128, 2], PV, tag="ones128", name="ones128")

        # ---- load + round everything (staging pool closes afterwards) -----
        with tc.tile_pool(name="stage", bufs=2) as stage:
            def load_row(name, dst):
                w = dst.shape[-1]
                s = stage.tile([1, N], F32, tag="rowstage", name="rowstage")
                nc.sync.dma_start(out=s[0:1, 0:w], in_=d[name][0:1, 0:w])
                nc.vector.tensor_copy(dst, s[0:1, 0:w])

            def load_w(lst, name, c, act=False):
                s = stage.tile([128, DM], F32, tag="wstage", name="wstage")
                nc.sync.dma_start(out=s, in_=d[name][c * 128 : (c + 1) * 128, :])
                if act:
                    nc.scalar.copy(lst[c], s)
                else:
                    nc.vector.tensor_copy(lst[c], s)

            def load_x(c, i):
                isl = bass.ts(i, IC)
                s = stage.tile([128, IC], F32, tag="xstage", name="xstage")
                nc.sync.dma_start(out=s, in_=d["xT"][c * 128 : (c + 1) * 128, isl])
                if c == 1:
                    nc.scalar.copy(xT[c][:, isl], s)
                else:
                    nc.vector.tensor_copy(xT[c][:, isl], s)

            # critical-path order: Wk + x slice 0 unblock the first K-proj
            for c in range(2):
                load_w(wk, "Wk", c)
            for c in range(2):
                load_x(c, 0)
            for c in range(2):
                load_w(wq, "Wq", c)
                nc.sync.dma_start(out=bqk[c], in_=d["bqk"][c * 128 : (c + 1) * 128, :])
            for i in range(1, NI):
                for c in range(2):
                    load_x(c, i)
            qrs = stage.tile([2, N], F32, tag="qrs", name="qrs")
            nc.sync.dma_start(out=qrs, in_=d["qrows"][:, :])
            krs = stage.tile([2, N], F32, tag="krs", name="krs")
            nc.sync.dma_start(out=krs, in_=d["krows"][:, :])
            for h in range(NH):
                nc.vector.tensor_copy(qT[h][64:66, :], qrs)
                nc.vector.tensor_copy(kT[h][64:66, :], krs)
            for c in range(2):
                load_w(wv, "Wv", c, act=True)
                load_w(wo, "Wo", c, act=True)
                nc.sync.dma_start(out=bo_sb[c], in_=d["bo"][c * 128 : (c + 1) * 128, :])

            s = stage.tile([1, N], F32, tag="rowstage", name="rowstage")
            nc.sync.dma_start(out=s[0:1, 0:DM], in_=d["bv"][0:1, :])
            nc.vector.tensor_copy(bv_r, s[0:1, 0:DM])
            nc.gpsimd.partition_broadcast(
                bv_bc[:, :, :].rearrange("p h k -> p (h k)"), bv_r
            )
            load_row("inval", inval_r)

            ones_stage = stage.tile([1, 128], F32, tag="ones", name="ones")
            nc.vector.memset(ones_stage, 1.0)
            nc.vector.tensor_copy(ones_col, ones_stage)
            o128s = stage.tile([128, 2], F32, tag="o128s", name="o128s")
            nc.vector.memset(o128s, 1.0)
            nc.vector.tensor_copy(ones128, o128s)


        groups = [list(range(g, min(g + EG, NJ))) for g in range(0, NJ, EG)]
        with (
            tc.tile_pool(name="psA", bufs=2, space="PSUM") as psA,
            tc.tile_pool(name="psS", bufs=3, space="PSUM") as psS,
            tc.tile_pool(name="expS", bufs=2) as expP,
            tc.tile_pool(name="nrm", bufs=2) as nrm,
            tc.tile_pool(name="attP", bufs=2) as attP,
            tc.tile_pool(name="outP", bufs=2) as outP,
        ):
            # ---- K then Q projections, i-outer so slice 0 unblocks fast ---
            def proj_kq(ws, i, col):
                isl = bass.ts(i, IC)
                for m in range(2):
                    p = psA.tile([128, IC], F32, tag="proj", name="proj")
                    for c in range(2):
                        nc.tensor.matmul(
                            p,
                            lhsT=ws[c][:, m * 128 : (m + 1) * 128],
                            rhs=xT[c][:, isl],
                            start=(c == 0),
                            stop=(c == 1),
                        )
                    dst = kT if col else qT
                    for hh in range(2):
                        h = 2 * m + hh
                        if hh:
                            nc.scalar.activation(
                                dst[h][0:64, isl],
                                p[hh * 64 : (hh + 1) * 64, :],
                                mybir.ActivationFunctionType.Identity,
                                bias=bqk[m][hh * 64 : (hh + 1) * 64, col : col + 1],
                            )
                        else:
                            nc.vector.tensor_scalar_add(
                                dst[h][0:64, isl],
                                p[hh * 64 : (hh + 1) * 64, :],
                                bqk[m][hh * 64 : (hh + 1) * 64, col : col + 1],
                            )

            for i in range(NI):
                proj_kq(wk, i, 1)
                proj_kq(wq, i, 0)
            for j in range(NJ):
                p = psA.tile([128, DM], F32, tag="proj", name="proj")
                jsl = bass.ts(j, 128)
                for c in range(2):
                    nc.tensor.matmul(
                        p,
                        lhsT=xT[c][:, jsl],
                        rhs=wv[c],
                        start=(c == 0),
                        stop=(c == 1),
                    )
                nc.vector.tensor_tensor(
                    vA[j][:, :, 0:DK],
                    p[:, :].rearrange("p (h k) -> p h k", h=NH),
                    bv_bc,
                    op=mybir.AluOpType.add,
                )
                nc.vector.tensor_copy(vA[j][:, :, DK:DKP], vones)

            # ---- vbar_aug = [mean_j V_h, 1.0] per head (uniform-row fix) --
            vp = psA.tile([2, NH, DKP], F32, tag="proj", name="vbarp")
            for j in range(NJ):
                nc.tensor.matmul(
                    vp[:, :, :],
                    lhsT=ones128,
                    rhs=vA[j][:, :, :],
                    start=(j == 0),
                    stop=(j == NJ - 1),
                )
            for h in range(NH):
                nc.vector.tensor_scalar_mul(vbar[h], vp[0:1, h, :], 1.0 / N)


            # ---- attention + output projection ----------------------------
            def out_proj(i, attT):
                isl = bass.ts(i, IC)
                for e in range(2):
                    p = psA.tile([128, IC], F32, tag="proj", name="outp")
                    for c in range(2):
                        nc.tensor.matmul(
                            p,
                            lhsT=wo[c][:, e * 128 : (e + 1) * 128],
                            rhs=attT[c],
                            start=(c == 0),
                            stop=(c == 1),
                        )
                    o = outP.tile([128, IC], F32, tag="out", name="out")
                    nc.vector.tensor_scalar_add(o, p, bo_sb[e])
                    nc.sync.dma_start(
                        out=d["outT"][e * 128 : (e + 1) * 128, isl], in_=o
                    )

            pending = None
            for i in range(NI):
                isl = bass.ts(i, IC)
                attT = [attP.tile([128, IC], MM, tag=f"attT{c}", name=f"attT{c}") for c in range(2)]
                for h in range(NH):
                    up = psA.tile([66, IC], F32, tag="proj", name="U")
                    for grp in groups:
                        g = len(grp)
                        sp = psS.tile([128, EG, IC], F32, tag="S", name="S")
                        for gg, j in enumerate(grp):
                            nc.tensor.matmul(
                                sp[:, gg, :],
                                lhsT=kT[h][:, bass.ts(j, 128)],
                                rhs=qT[h][:, isl],
                                start=True,
                                stop=True,
                            )
                        e = expP.tile([128, EG, IC], PV, tag="expS", name="expS")
                        nc.scalar.activation(
                            e[:, 0:g, :], sp[:, 0:g, :], Exp, scale=SCALE
                        )
                        for gg, j in enumerate(grp):
                            nc.tensor.matmul(
                                up,
                                lhsT=vA[j][:, h, :],
                                rhs=e[:, gg, :],
                                start=(j == 0),
                                stop=False,
                            )
                    nc.tensor.matmul(
                        up,
                        lhsT=vbar[h],
                        rhs=inval_r[0:1, isl],
                        start=False,
                        stop=True,
                    )
                    rsum = nrm.tile([1, IC], F32, tag="rsum", name="rsum")
                    nc.vector.tensor_copy(rsum, up[64:65, :])
                    rec = nrm.tile([1, IC], F32, tag="rec", name="rec")
                    nc.vector.reciprocal_approx_fast(rec, rsum)
                    bc = nrm.tile([64, IC], F32, tag="bc", name="bc")
                    nc.gpsimd.partition_broadcast(bc, rec[0:1, :])
                    nc.vector.tensor_mul(
                        attT[h // 2][(h % 2) * 64 : (h % 2 + 1) * 64, :],
                        up[0:64, :],
                        bc,
                    )
                if pending is not None:
                    out_proj(*pending)
                pending = (i, attT)
            out_proj(*pending)


_NC_CACHE = {}


def _build():
    key = USE_F32R
    if key in _NC_CACHE:
        return _NC_CACHE[key]
    nc = bacc.Bacc("TRN2", debug=False, num_devices=B)
    d = {
        "xT": nc.dram_tensor("xT", [DIN, N], F32, kind="ExternalInput").ap(),
        "Wq": nc.dram_tensor("Wq", [DIN, DM], F32, kind="ExternalInput").ap(),
        "Wk": nc.dram_tensor("Wk", [DIN, DM], F32, kind="ExternalInput").ap(),
        "Wv": nc.dram_tensor("Wv", [DIN, DM], F32, kind="ExternalInput").ap(),
        "Wo": nc.dram_tensor("Wo", [DM, DM], F32, kind="ExternalInput").ap(),
        "bqk": nc.dram_tensor("bqk", [DM, 2], F32, kind="ExternalInput").ap(),
        "bv": nc.dram_tensor("bv", [1, DM], F32, kind="ExternalInput").ap(),
        "bo": nc.dram_tensor("bo", [DM, 1], F32, kind="ExternalInput").ap(),
        "qrows": nc.dram_tensor("qrows", [2, N], F32, kind="ExternalInput").ap(),
        "krows": nc.dram_tensor("krows", [2, N], F32, kind="ExternalInput").ap(),
        "inval": nc.dram_tensor("inval", [1, N], F32, kind="ExternalInput").ap(),
        "outT": nc.dram_tensor("outT", [DM, N], F32, kind="ExternalOutput").ap(),
    }
    with TileContext(nc) as tc:
        _emit(nc, tc, d)
    nc.compile()
    _NC_CACHE[key] = nc
    return nc


def _host_marshal(x, attention_mask, Wq, bq, Wk, bk, Wv, bv, Wo, bo):
    x = np.asarray(x, dtype=np.float32)
    m = np.asarray(attention_mask).astype(bool)
    pos = np.arange(N)
    start = m.argmax(axis=1)  # first True index
    end = N - 1 - m[:, ::-1].argmax(axis=1)  # last True index (exclusive bound)
    valid = (pos[None, :] >= start[:, None]) & (pos[None, :] < end[:, None])
    valid_f = valid.astype(np.float32)
    vbias_f = np.where(valid, np.float32(0.0), np.float32(NEG)).astype(np.float32)

    common = {
        "Wq": np.ascontiguousarray(Wq, dtype=np.float32),
        "Wk": np.ascontiguousarray(Wk, dtype=np.float32),
        "Wv": np.ascontiguousarray(Wv, dtype=np.float32),
        "Wo": np.ascontiguousarray(Wo, dtype=np.float32),
        "bqk": np.ascontiguousarray(
            np.stack([np.asarray(bq), np.asarray(bk)], axis=1), dtype=np.float32
        ),
        "bv": np.asarray(bv, dtype=np.float32).reshape(1, DM),
        "bo": np.asarray(bo, dtype=np.float32).reshape(DM, 1),
    }
    in_maps = []
    for b in range(B):
        im = dict(common)
        im["xT"] = np.ascontiguousarray(x[b].T)
        inval = np.float32(1.0) - valid_f[b : b + 1]
        im["qrows"] = np.concatenate([valid_f[b : b + 1], inval], axis=0)
        im["krows"] = np.concatenate(
            [vbias_f[b : b + 1], np.full((1, N), NEG, dtype=np.float32)], axis=0
        )
        im["inval"] = inval
        in_maps.append(im)
    return in_maps


def kernel(x, attention_mask, Wq, bq, Wk, bk, Wv, bv, Wo, bo, _trace=False):
    nc = _build()
    in_maps = _host_marshal(x, attention_mask, Wq, bq, Wk, bk, Wv, bv, Wo, bo)
    res = bass_utils.run_bass_kernel_spmd(
        nc, in_maps, core_ids=list(range(B)), trace=_trace
    )
    out = np.stack([np.ascontiguousarray(r["outT"].T) for r in res.results], axis=0)
    if _trace:
        kernel.last_exec_time_ns = res.exec_time_ns
        kernel.last_results = res
    return out
